# revision 6
# baseline (speedup 1.0000x reference)
"""Trainium2 Bass kernel: sliding-window multi-head attention with ALiBi.

Reference computation (B=2, S=4096, E=512, H=8, D=64, window 513):
    q = (inputs_q @ w_q);  k = (inputs_kv @ w_k);  v = (inputs_kv @ w_v)
    att = softmax(q k^T / 8 + alibi, sliding window +-256)
    out = (att v) @ w_o

Sharding: 8 cores = 2 batches x 4 sequence quarters (1024 q rows per core).
Each core gets its kv slice with a 256-row halo (zero-padded at sequence
edges).  All cores run the identical program (SPMD); edge handling is pure
data:
  - zero-padded X_kv makes K=V=0 on out-of-range rows,
  - a host-provided validity column appended to V makes the softmax
    denominator (accumulated by the same AV matmul) skip those rows,
  - the window/ALiBi mask is applied as a multiplicative exp-mask after
    exp(): P = exp(S) * G, where G[h] is Toeplitz in (kv - q) and therefore
    shared by all query blocks (G = exp(-slope_h * |rel|) * [|rel| <= 256]);
    the host pre-unrolls it into the 6-chunk score layout.

v4: whole data path in bfloat16 (same 1 cycle/row matmul rate as f32r,
half the DMA, 2x packed DVE mask-multiply), exp reads PSUM f32 and
writes bf16 directly; score chunks 0 and 5 carry only their live q-half
(the other half is provably outside the window), packing each score
tile to 1280 used columns with bank-aligned start=True chunks - 17%
less PE and ACT work in attention; input DMAs are column-striped across
queues and emitted in just-in-time order, G masks load once like
weights; the AV matmul + normalize of each (qb, h) pair is emitted one
pair late so PE always has the next score matmuls in hand; the
normalize is reciprocal (DVE) + partition-broadcast (GPSIMD's only op,
no DSP library switching) + multiply (DVE).

Layout: scores are computed transposed (S^T[kv, q]) so the AV matmul needs
no on-chip transposes: lhsT = [V | valid], rhs = P^T gives O^T[d, q] plus
the denominator row.  All 6 kv-chunks of a q-block live in one [128,1536]
PSUM tile (3 banks; the even chunk of each bank carries start=True, which
zeroes the whole 2KB zero-region, so the odd chunk runs start=False).
Normalization: reciprocal of the denominator row is partition-broadcast on
GPSIMD and multiplied in on DVE.  The final projection consumes O^T tiles
directly as stationary operands.
"""

import sys

if "/opt/trn_rl_repo" not in sys.path:
    sys.path.insert(0, "/opt/trn_rl_repo")

import numpy as np

import concourse.bacc as bacc
import concourse.mybir as mybir
import concourse.tile as tile
from concourse.bass_utils import run_bass_kernel_spmd

# ---------------------------------------------------------------- geometry
B, S, E = 2, 4096, 512
H, D = 8, 64
HD = H * D              # 512
HALF = 256              # window half-width (ATTENTION_WINDOW=512 -> 513 wide)
NCORES = 8
SQ = 4                  # sequence shards per batch
QROWS = S // SQ         # 1024 q rows per core
KVROWS = QROWS + 2 * HALF   # 1536 kv rows per core (with halo)
QB = 4                  # q blocks per core
QBLK = QROWS // QB      # 256 q cols per block
NCH = 6                 # kv chunks per q block
CBLK = 128              # kv chunk rows
SP6 = NCH * QBLK        # 1536: 3 PSUM banks per score tile
SP5 = 1280              # used score columns (c0/c5 carry only their live q-half)
# score-column layout: (chunk, col_off, width, q_off).  Chunk 0 is dead for
# q >= 128 and chunk 5 for q < 128 (outside the +-256 window), so each
# contributes only its live half; offsets 0/512/1024 sit at PSUM bank
# starts and carry start=True (which zeroes the whole 2KB bank).
CLAY = [(1, 0, 256, 0), (0, 256, 128, 0), (5, 384, 128, 128),
        (2, 512, 256, 0), (3, 768, 256, 0), (4, 1024, 256, 0)]

import os
WSPL = int(os.environ.get("K_WSPL", "2"))    # DMA splits per weight tensor
XSPL = int(os.environ.get("K_XSPL", "4"))    # DMA splits for xq
KVSPL = int(os.environ.get("K_KVSPL", "6"))  # DMA splits for xkv

F32 = mybir.dt.float32
BF16 = mybir.dt.bfloat16

_CACHE = {}


def _build_program(repeats=1):
    """Build + compile the SPMD program (cached per process).

    repeats > 1 re-runs the whole computation that many times (same inputs,
    same outputs) - used only for wall-clock HW timing by difference.
    """
    key = ("nc", repeats)
    if key in _CACHE:
        return _CACHE[key]

    nc = bacc.Bacc("TRN2", target_bir_lowering=False, debug=False,
                   enable_asserts=True)

    xq_d = nc.dram_tensor("xqT", [E, QROWS], BF16, kind="ExternalInput")
    xkv_d = nc.dram_tensor("xkvT", [E, KVROWS], BF16, kind="ExternalInput")
    wq_d = nc.dram_tensor("wq", [E, HD], BF16, kind="ExternalInput")
    wk_d = nc.dram_tensor("wk", [E, HD], BF16, kind="ExternalInput")
    wv_d = nc.dram_tensor("wv", [E, HD], BF16, kind="ExternalInput")
    wo_d = nc.dram_tensor("wo", [HD, E], BF16, kind="ExternalInput")
    g_d = nc.dram_tensor("gmask", [H, 128, SP5], BF16, kind="ExternalInput")
    val_d = nc.dram_tensor("validc", [128, KVROWS // CBLK], F32,
                           kind="ExternalInput")
    y_d = nc.dram_tensor("y", [QROWS, E], BF16, kind="ExternalOutput")

    EXP = mybir.ActivationFunctionType.Exp

    with tile.TileContext(nc) as tc:
        with (
            tc.tile_pool(name="wts", bufs=1) as wts,
            tc.tile_pool(name="bigx", bufs=2) as bigx,
            tc.tile_pool(name="gp", bufs=1) as gp,
            tc.tile_pool(name="proj", bufs=1) as proj,
            tc.tile_pool(name="pwork", bufs=4) as pwork,
            tc.tile_pool(name="small", bufs=4) as small,
            tc.tile_pool(name="bigp", bufs=2, space="PSUM") as bigp,
            tc.tile_pool(name="otp", bufs=2, space="PSUM") as otp,
        ):
            # ---- load weights: one DMA per tensor into a single wide tile
            # ([512, C] viewed as [4, 128, C] -> [128, 4, C]); slice per e.
            # Real DMA queues give one engine (~22 GB/s) per in-flight
            # copy, so large tensors are split into several column-striped
            # DMAs that land on different queues and run concurrently.
            def load_split(pool, tag, dram_ap, cols, nspl):
                t = pool.tile([128, 4 * cols], BF16, tag=tag, name=tag)
                tv = t[:].rearrange("p (e c) -> p e c", c=cols)
                sv = dram_ap.rearrange("(e p) c -> p e c", p=128)
                w = cols // nspl
                for i in range(nspl):
                    nc.sync.dma_start(tv[:, :, w * i:w * (i + 1)],
                                      sv[:, :, w * i:w * (i + 1)])
                return [t[:, cols * e:cols * (e + 1)] for e in range(4)]

            def load1(dram, name, cols):
                return load_split(wts, name, dram.ap(), cols, WSPL)

            wq_sb = load1(wq_d, "wq", HD)

            # ---- persistent activation tiles
            qt_sb = [proj.tile([128, QROWS], BF16, tag=f"qt{t}", name=f"qt{t}")
                     for t in range(4)]
            kt_sb = [proj.tile([128, KVROWS], BF16, tag=f"kt{t}",
                               name=f"kt{t}") for t in range(4)]
            # V tiles: head h occupies cols [65h, 65h+64), col 65h+64 = valid
            v_sb = [proj.tile([128, 65 * H], BF16, tag=f"v{b}", name=f"v{b}")
                    for b in range(KVROWS // CBLK)]
            ot_sb = [proj.tile([128, QROWS], BF16, tag=f"ot{t}", name=f"ot{t}")
                     for t in range(4)]

            for rep in range(repeats):
                # DMA emission order = service order: interleave the
                # (once-only) weight loads with the inputs so each
                # projection's operands land just in time:
                # wq | xq | wk wv | xkv | valid wo | g.
                xq_sb = load_split(bigx, "bigxq", xq_d.ap(), QROWS, XSPL)

                if rep == 0:
                    wk_sb = load1(wk_d, "wk", HD)
                    wv_sb = load1(wv_d, "wv", HD)

                xkv_sb = load_split(bigx, "bigxkv", xkv_d.ap(), KVROWS,
                                    KVSPL)

                if rep == 0:
                    valid_sb = small.tile([128, KVROWS // CBLK], F32,
                                          tag="validc", name="validc")
                    nc.sync.dma_start(valid_sb[:], val_d.ap()[:])
                    ones8 = small.tile([128, H], F32, tag="ones8",
                                       name="ones8")
                    nc.vector.memset(ones8[:], 1.0)
                    wo_sb = load1(wo_d, "wo", E)
                    # G masks are input-independent constants - loaded once,
                    # like the weights (h descending = consumption order)
                    g_sb = [None] * H
                    for h in range(H - 1, -1, -1):
                        t = gp.tile([128, SP5], BF16, tag=f"g{h}",
                                    name=f"g{h}")
                        nc.sync.dma_start(t[:], g_d.ap()[h])
                        g_sb[h] = t

                # ---- projections (n outer: the first column-half DMA is
                # enough to start; t descending inside)
                for n in range(QROWS // 512):
                    for t in range(3, -1, -1):
                        ps = bigp.tile([128, SP6], F32, tag="bigp",
                                       name="psq")
                        for e in range(4):
                            nc.tensor.matmul(
                                ps[:, :512],
                                lhsT=wq_sb[e][:, 128 * t:128 * (t + 1)],
                                rhs=xq_sb[e][:, 512 * n:512 * (n + 1)],
                                start=(e == 0), stop=(e == 3))
                        nc.scalar.copy(
                            qt_sb[t][:, 512 * n:512 * (n + 1)],
                            ps[:, :512])

                for n in range(KVROWS // 512):
                    for t in range(3, -1, -1):
                        ps = bigp.tile([128, SP6], F32, tag="bigp",
                                       name="psk")
                        for e in range(4):
                            nc.tensor.matmul(
                                ps[:, :512],
                                lhsT=wk_sb[e][:, 128 * t:128 * (t + 1)],
                                rhs=xkv_sb[e][:, 512 * n:512 * (n + 1)],
                                start=(e == 0), stop=(e == 3))
                        nc.scalar.copy(
                            kt_sb[t][:, 512 * n:512 * (n + 1)],
                            ps[:, :512])

                for blk in range(KVROWS // CBLK):
                    ps = bigp.tile([128, SP6], F32, tag="bigp", name="psv")
                    for e in range(4):
                        nc.tensor.matmul(
                            ps[:, :512],
                            lhsT=xkv_sb[e][:, 128 * blk:128 * (blk + 1)],
                            rhs=wv_sb[e][:],
                            start=(e == 0), stop=(e == 3))
                    vv = v_sb[blk][:].rearrange("p (h c) -> p h c", c=65)
                    nc.scalar.copy(
                        vv[:, :, 0:64],
                        ps[:, :512].rearrange("p (h c) -> p h c", c=64))
                    nc.vector.tensor_scalar_mul(
                        vv[:, :, 64], ones8[:],
                        valid_sb[:, blk:blk + 1])

                # ---- attention (qb outer, h descending).  The AV matmul +
                # normalization of each pair is emitted one pair late so PE
                # can issue the next pair's score matmuls while ACT/DVE chew
                # on exp/mask of the current one.
                def finish_pair(qb, h, pm6):
                    th, ph = h // 2, 64 * (h % 2)
                    ot = otp.tile([65, QBLK], F32, tag="otp", name="ot")
                    for i, (c, off, w, qo) in enumerate(CLAY):
                        nc.tensor.matmul(
                            ot[:, qo:qo + w],
                            lhsT=v_sb[2 * qb + c][:, 65 * h:65 * h + 65],
                            rhs=pm6[:, off:off + w],
                            start=(i == 0), stop=(i == len(CLAY) - 1),
                            skip_group_check=(i > 0))
                    # reciprocal of the denominator row, partition-broadcast
                    # on GPSIMD (its only op - no gpsimd library switching),
                    # multiply in on DVE
                    rec = small.tile([1, QBLK], F32, tag="rec", name="rec")
                    nc.vector.reciprocal(rec[:], ot[64:65, :])
                    bc = pwork.tile([64, QBLK], F32, tag="bc", name="bc")
                    nc.gpsimd.partition_broadcast(bc[:], rec[:])
                    nc.vector.tensor_mul(
                        ot_sb[th][ph:ph + 64, QBLK * qb:QBLK * (qb + 1)],
                        ot[0:64, :], bc[:])

                # Output projection of one q block (2 row-blocks of 128);
                # interleaved into the attention stream as soon as the
                # block's last head is normalized, so there is no serial
                # projection tail after the attention loop.
                def outproj(qb):
                    for yb in (2 * qb, 2 * qb + 1):
                        yp = bigp.tile([128, SP6], F32, tag="bigp",
                                       name="yp")
                        for t in range(4):
                            nc.tensor.matmul(
                                yp[:, :512],
                                lhsT=ot_sb[t][:, 128 * yb:128 * (yb + 1)],
                                rhs=wo_sb[t][:],
                                start=(t == 0), stop=(t == 3))
                        ystage = pwork.tile([128, 512], BF16, tag="ys",
                                            name="ystage")
                        nc.scalar.copy(ystage[:], yp[:, :512])
                        nc.sync.dma_start(
                            y_d.ap()[128 * yb:128 * (yb + 1), :],
                            ystage[:])

                pending = None
                for qb in range(QB):
                    for h in range(H - 1, -1, -1):
                        th, ph = h // 2, 64 * (h % 2)
                        qs = qt_sb[th][ph:ph + 64, QBLK * qb:QBLK * (qb + 1)]
                        sp = bigp.tile([128, SP6], F32, tag="bigp", name="sp6")
                        for c, off, w, qo in CLAY:
                            k0 = QBLK * qb + CBLK * c
                            bank0 = off % 512 == 0
                            nc.tensor.matmul(
                                sp[:, off:off + w],
                                lhsT=kt_sb[th][ph:ph + 64, k0:k0 + CBLK],
                                rhs=qs[:, qo:qo + w],
                                start=bank0, stop=True,
                                skip_group_check=not bank0)
                        if pending is not None:
                            finish_pair(*pending)
                        pe6 = pwork.tile([128, SP5], BF16, tag="pw",
                                         name="pe6")
                        nc.scalar.activation(pe6[:], sp[:, :SP5], EXP)
                        pm6 = pwork.tile([128, SP5], BF16, tag="pw",
                                         name="pm6")
                        nc.vector.tensor_mul(pm6[:], pe6[:], g_sb[h][:])
                        pending = (qb, h, pm6)
                finish_pair(*pending)
                for qb in range(QB):
                    outproj(qb)

    nc.compile()
    _CACHE[key] = nc
    return nc


def build_in_maps(inputs_q, inputs_kv, w_q, w_k, w_v, w_o):
    """Host-side sharding: slice/transpose/pad per core + mask tensors."""
    np_bf = mybir.dt.np(BF16)
    inputs_q = np.asarray(inputs_q, np.float32)
    inputs_kv = np.asarray(inputs_kv, np.float32)

    wq = np.ascontiguousarray(np.asarray(w_q, np.float32) * 0.125).astype(np_bf)
    wk = np.ascontiguousarray(np.asarray(w_k, np.float32)).astype(np_bf)
    wv = np.ascontiguousarray(np.asarray(w_v, np.float32)).astype(np_bf)
    wo = np.ascontiguousarray(np.asarray(w_o, np.float32)).astype(np_bf)

    # Toeplitz exp-mask, pre-unrolled into the packed score layout:
    # chunk c, kv row r, q col i -> rel = i - r - 128c + 256
    slopes = np.array([2.0 ** (-(i + 1)) for i in range(H)], np.float64)
    r = np.arange(128)[:, None]
    i = np.arange(QBLK)[None, :]
    g = np.empty((H, 128, SP5), np_bf)
    for c, off, w, qo in CLAY:
        rel = i[:, qo:qo + w] - r - 128 * c + 256
        band = (np.abs(rel) <= HALF)
        for h in range(H):
            g[h, :, off:off + w] = (
                np.exp(-slopes[h] * np.abs(rel)) * band).astype(np_bf)

    in_maps = []
    for c in range(NCORES):
        b, sq = divmod(c, SQ)
        g0 = QROWS * sq
        xq = np.ascontiguousarray(inputs_q[b, g0:g0 + QROWS, :].T).astype(np_bf)
        kvlo = g0 - HALF
        lo, hi = max(0, kvlo), min(S, g0 + QROWS + HALF)
        xkv = np.zeros((E, KVROWS), np_bf)
        xkv[:, lo - kvlo:hi - kvlo] = inputs_kv[b, lo:hi, :].T.astype(np_bf)
        valid = np.zeros((KVROWS,), np.float32)
        valid[lo - kvlo:hi - kvlo] = 1.0
        validc = np.ascontiguousarray(valid.reshape(KVROWS // CBLK, CBLK).T)
        in_maps.append({
            "xqT": xq, "xkvT": xkv,
            "wq": wq, "wk": wk, "wv": wv, "wo": wo,
            "gmask": g, "validc": validc,
        })
    return in_maps


def assemble_output(results):
    out = np.empty((B, S, E), np.float32)
    for c in range(NCORES):
        b, sq = divmod(c, SQ)
        out[b, QROWS * sq:QROWS * (sq + 1), :] = (
            results[c]["y"].astype(np.float32))
    return out


def kernel(inputs_q, inputs_kv, w_q, w_k, w_v, w_o):
    nc = _build_program()
    in_maps = build_in_maps(inputs_q, inputs_kv, w_q, w_k, w_v, w_o)
    res = run_bass_kernel_spmd(nc, in_maps, core_ids=list(range(NCORES)))
    return assemble_output(res.results)
